# revision 35
# baseline (speedup 1.0000x reference)
"""Trainium2 Bass kernel for the vq_codebook problem (nn_GSP_37890201485791).

Data-parallel over batch across 8 NeuronCores; params replicated.
Self-contained: shapes hardcoded, no sibling imports.
"""
import os
import numpy as np
from contextlib import ExitStack

B, C, H, W = 128, 2048, 14, 14
N = H * W               # 196
D, K, NCLS = 512, 64, 100
MU, EPS = 0.3, 0.1
NCORES = 8
BPC = B // NCORES       # 16 batches per core
NPAIR = BPC // 2        # 8 pairs per core
N2 = 2 * N              # 392
CT = C // 128           # 16 channel chunks
DT = D // 128           # 4 embed chunks
CN = float((1.0 - MU) * N)
NEWTON_ITERS = 8

_cache = {}


def _register_mul_max():
    """Author a fused (in0*in1, max-reduce) custom-DVE op at runtime.

    The ISA TensorTensorReduce opcode faults on this runtime; the
    custom-DVE table path (same one reciprocal_approx uses) works.
    """
    import concourse.dve_ops as dvo
    from concourse.dve_spec import Spec, Src0, Src1, MaxNeg, maxx, lower
    from concourse.dve_uop import DveOpSpec

    name = "ANT_MUL_MAX_REDUCE"
    if name in dvo._SUB_OPCODE_FOR_NAME:
        return next(op for op in dvo.OPS if op.name == name)
    def _ref(in0, in1, c0, c1, c2):
        b = (in0.astype(np.float32) * in1).astype(np.float32)
        return b, b.reshape(b.shape[0], -1).max(axis=-1, keepdims=True)

    spec = Spec(body=Src0 * Src1, accum=maxx, accum_init=MaxNeg, reference=_ref)
    opcode = dvo._CUSTOM_DVE_ROW_BASE + len(dvo.OPS)
    assert opcode < 0x20
    shas = {}
    for ver in ("v3", "v4"):
        u = lower(spec, ver=ver)
        shas[ver] = DveOpSpec(name=name, opcode=opcode, uops=u, rd1_en=True).sha(ver)
    op = dvo.DveOp(name, spec, subdim=False, uops_sha=shas)
    dvo.OPS.append(op)
    dvo._SUB_OPCODE_FOR_NAME[name] = opcode
    dvo.CUSTOM_DVE_SPECS[name] = spec
    return op


def _pin_act_tables():
    """Force every activation we use into one table set so the ACT engine
    loads its spline tables exactly once (Exp/Ln otherwise alternate sets)."""
    import concourse.bacc as bc
    import concourse.hw_specs as hws
    from concourse import mybir

    if getattr(bc, "_ant_act_pin", False):
        return
    F = mybir.ActivationFunctionType
    mine = {F.Copy, F.Identity, F.Exp, F.Ln, F.Square}
    keep = "natural_log_exp_and_others"
    orig = hws.get_activation_tables

    def patched(arch):
        real = orig(arch)
        return {name: (fns if name == keep else fns - mine)
                for name, fns in real.items()}

    bc.get_activation_tables = patched
    bc._ant_act_pin = True


def _build_nc():
    import concourse.tile as tile
    from concourse import bacc, mybir
    from concourse.masks import make_identity

    MULMAX = _register_mul_max()
    _pin_act_tables()

    f32 = mybir.dt.float32
    A = mybir.AluOpType
    F = mybir.ActivationFunctionType
    AX = mybir.AxisListType

    nc = bacc.Bacc("TRN2", target_bir_lowering=False, debug=False,
                   num_devices=NCORES)

    bf16 = mybir.dt.bfloat16
    feat_d = nc.dram_tensor("features", [BPC, C, N], f32, kind="ExternalInput").ap()
    fhi_d = nc.dram_tensor("fhi", [BPC, C, N], bf16, kind="ExternalInput").ap()
    flo_d = nc.dram_tensor("flo", [BPC, C, N], bf16, kind="ExternalInput").ap()
    whi_d = nc.dram_tensor("ewThi", [C, D], bf16, kind="ExternalInput").ap()
    wlo_d = nc.dram_tensor("ewTlo", [C, D], bf16, kind="ExternalInput").ap()
    ebrow_d = nc.dram_tensor("ebrow", [1, D], f32, kind="ExternalInput").ap()
    cwT_d = nc.dram_tensor("cwT", [D, NCLS], f32, kind="ExternalInput").ap()
    cbrow_d = nc.dram_tensor("cbrow", [1, NCLS], f32, kind="ExternalInput").ap()
    proto_d = nc.dram_tensor("prototypes", [K, D], f32, kind="ExternalInput").ap()

    probs_d = nc.dram_tensor("x_probs", [BPC, NCLS], f32, kind="ExternalOutput").ap()
    emb_d = nc.dram_tensor("x_emb", [BPC, D], f32, kind="ExternalOutput").ap()
    attr_d = nc.dram_tensor("x_attr", [BPC, K], f32, kind="ExternalOutput").ap()

    with tile.TileContext(nc) as tc, ExitStack() as ctx:
        singles = ctx.enter_context(tc.tile_pool(name="singles", bufs=1))
        featp = ctx.enter_context(tc.tile_pool(name="featp", bufs=3))
        bowp = ctx.enter_context(tc.tile_pool(name="bowp", bufs=5))
        small = ctx.enter_context(tc.tile_pool(name="small", bufs=4))
        scr = ctx.enter_context(tc.tile_pool(name="scr", bufs=2))
        ps_bow = ctx.enter_context(tc.tile_pool(name="ps_bow", bufs=4, space="PSUM"))
        ps1 = ctx.enter_context(tc.tile_pool(name="ps1", bufs=4, space="PSUM"))

        # ---------- constants / params ----------
        whi_sb = singles.tile([128, CT, D], bf16)
        wlo_sb = singles.tile([128, CT, D], bf16)
        for ci in range(CT):
            nc.sync.dma_start(out=whi_sb[:, ci, :], in_=whi_d[ci * 128:(ci + 1) * 128, :])
            nc.sync.dma_start(out=wlo_sb[:, ci, :], in_=wlo_d[ci * 128:(ci + 1) * 128, :])
        cwT_sb = singles.tile([128, DT, NCLS], f32)
        for j in range(DT):
            nc.sync.dma_start(out=cwT_sb[:, j, :], in_=cwT_d[j * 128:(j + 1) * 128, :])
        ebrow_sb = singles.tile([1, D], f32)
        nc.sync.dma_start(out=ebrow_sb[:], in_=ebrow_d)
        cbrow_sb = singles.tile([1, NCLS], f32)
        nc.sync.dma_start(out=cbrow_sb[:], in_=cbrow_d)

        ident = singles.tile([128, 128], f32)
        make_identity(nc, ident[:])
        ones_col = singles.tile([128, 1], f32)
        nc.vector.memset(ones_col[:], 1.0)
        ones_row = singles.tile([1, 128], f32)
        nc.vector.memset(ones_row[:], 1.0)
        bias_m20 = singles.tile([128, 1], f32)
        nc.vector.memset(bias_m20[:], -20.0)
        ones392 = singles.tile([1, N2], f32)
        nc.vector.memset(ones392[:], 1.0)

        # prototypes -> normalized, transposed  phatT [128, DT, K]
        proto_sb = singles.tile([K, D], f32)
        nc.sync.dma_start(out=proto_sb[:], in_=proto_d)
        psq = scr.tile([K, D], f32, tag="psq", bufs=1)
        pssq = small.tile([K, 1], f32, tag="pssq")
        nc.vector.affine_mul_reduce(out=psq[:], accum_out=pssq[:], in0=proto_sb[:],
                                    in1=proto_sb[:], scale=1.0, bias=0.0)
        plog = small.tile([K, 1], f32, tag="plog")
        nc.scalar.activation(plog[:], pssq[:], F.Ln)
        pinv = small.tile([K, 1], f32, tag="pinv")
        nc.scalar.activation(pinv[:], plog[:], F.Exp, scale=-0.5)
        nc.vector.tensor_scalar_min(pinv[:], pinv[:], 1e12)
        phat = singles.tile([K, D], f32)
        nc.vector.tensor_scalar_mul(phat[:], proto_sb[:], pinv[:])
        phatT = singles.tile([128, DT, K], f32)
        for j in range(DT):
            tp = ps1.tile([128, K], f32, tag="ps1")
            nc.tensor.transpose(tp[:], phat[:, j * 128:(j + 1) * 128],
                                ident[0:K, 0:K])
            nc.scalar.activation(phatT[:, j, :], tp[:], F.Copy)

        # persistent state
        gmp = singles.tile([128, CT, BPC], f32)
        attr_all = singles.tile([K, BPC], f32)

        # ---------- Pass A ----------
        for p in range(NPAIR):
            fthi = featp.tile([128, CT, 2, N], bf16, tag="fthi", bufs=2)
            ftlo = featp.tile([128, CT, 2, N], bf16, tag="ftlo", bufs=2)
            for b2 in range(2):
                nc.sync.dma_start(
                    out=fthi[:, :, b2, :],
                    in_=fhi_d[2 * p + b2].rearrange("(ct q) n -> q ct n", q=128))
                nc.sync.dma_start(
                    out=ftlo[:, :, b2, :],
                    in_=flo_d[2 * p + b2].rearrange("(ct q) n -> q ct n", q=128))

            bows = []
            for d in range(DT):
                bps = ps_bow.tile([128, N2], f32, tag="bowps")
                # bias: eb[d-chunk] outer-product ones, then accumulate conv
                nc.tensor.matmul(bps[:], lhsT=ebrow_sb[0:1, d * 128:(d + 1) * 128],
                                 rhs=ones392[:], start=True, stop=False)
                for ci in range(CT):
                    ds_ = slice(d * 128, (d + 1) * 128)
                    with tc.tile_critical():
                        nc.tensor.matmul(bps[:], lhsT=whi_sb[:, ci, ds_],
                                         rhs=fthi[:, ci, :, :], start=False,
                                         stop=False)
                        i2 = nc.tensor.matmul(bps[:], lhsT=whi_sb[:, ci, ds_],
                                              rhs=ftlo[:, ci, :, :], start=False,
                                              stop=False)
                        i2.ldweights = False
                    nc.tensor.matmul(bps[:], lhsT=wlo_sb[:, ci, ds_],
                                     rhs=fthi[:, ci, :, :], start=False,
                                     stop=(ci == CT - 1))
                bow_sb = bowp.tile([128, N2], f32, tag="bow")
                nc.scalar.activation(bow_sb[:], bps[:], F.Copy)
                bows.append(bow_sb)

            gps = ps1.tile([K, N2], f32, tag="ps1")
            for d in range(DT):
                nc.tensor.matmul(gps[:], lhsT=phatT[:, d, :], rhs=bows[d][:],
                                 start=(d == 0), stop=(d == DT - 1))
            sps = ps1.tile([1, N2], f32, tag="ps1")
            for d in range(DT):
                sq_sb = scr.tile([128, N2], f32, tag="sq")
                nc.vector.tensor_mul(sq_sb[:], bows[d][:], bows[d][:])
                nc.tensor.matmul(sps[:], lhsT=ones_col[:], rhs=sq_sb[:],
                                 start=(d == 0), stop=(d == DT - 1))
            lssq = small.tile([1, N2], f32, tag="lssq", bufs=2)
            nc.scalar.activation(lssq[:], sps[:], F.Ln)
            rinv = small.tile([1, N2], f32, tag="rinv", bufs=2)
            nc.scalar.activation(rinv[:], lssq[:], F.Exp, scale=-0.5)
            rbc = ps1.tile([K, N2], f32, tag="ps1")
            nc.tensor.matmul(rbc[:], lhsT=ones_row[0:1, 0:K], rhs=rinv[:],
                             start=True, stop=True)
            rbc_sb = scr.tile([K, N2], f32, tag="rbcsb", bufs=1)
            nc.scalar.activation(rbc_sb[:], rbc[:], F.Copy)
            tcos = scr.tile([K, N2], f32, tag="tcos", bufs=1)
            nc.vector.tensor_mul(tcos[:], gps[:], rbc_sb[:])
            kk0 = scr.tile([K, N2], f32, tag="kk0", bufs=1)
            nc.scalar.activation(kk0[:], tcos[:], F.Exp, scale=20.0,
                                 bias=bias_m20[0:K, :])
            KKp = scr.tile([K, N2], f32, tag="KKp", bufs=2)
            nc.vector.tensor_scalar_min(KKp[:], kk0[:], 1.0)
            eps_ = ps1.tile([1, N2], f32, tag="ps1")
            nc.tensor.matmul(eps_[:], lhsT=ones_col[0:K, :], rhs=KKp[:],
                             start=True, stop=True)
            eps_sb = small.tile([1, N2], f32, tag="eps_sb", bufs=2)
            nc.scalar.activation(eps_sb[:], eps_[:], F.Copy)
            Epair = small.tile([2, N], f32, tag="Epair", bufs=2)
            for b2 in range(2):
                nc.gpsimd.dma_start(out=Epair[b2:b2 + 1, :],
                                    in_=eps_sb[0:1, b2 * N:(b2 + 1) * N])

            # --- Sinkhorn: Newton from below, V0 = 0 (per pair, 2 rows) ---
            V = small.tile([2, 1], f32, tag="V")
            nc.vector.memset(V[:], 0.0)
            tE = scr.tile([2, N], f32, tag="tE", bufs=2)
            rec = scr.tile([2, N], f32, tag="rec", bufs=2)
            rscr = scr.tile([2, N], f32, tag="rscr", bufs=2)
            sqd = scr.tile([2, N], f32, tag="sqd", bufs=2)
            for it in range(NEWTON_ITERS):
                nc.vector.tensor_scalar_add(tE[:], Epair[:], V[:])
                nc.vector.reciprocal_approx_accurate(rec[:], tE[:], rscr[:])
                S = small.tile([2, 1], f32, tag="S")
                nc.vector.tensor_reduce(S[:], rec[:], axis=AX.X, op=A.add)
                S2 = small.tile([2, 1], f32, tag="S2")
                nc.vector.affine_mul_reduce(out=sqd[:], accum_out=S2[:], in0=rec[:],
                                            in1=rec[:], scale=1.0, bias=0.0)
                a = small.tile([2, 1], f32, tag="a")
                nc.vector.tensor_mul(a[:], V[:], S2[:])
                den = small.tile([2, 1], f32, tag="den")
                nc.vector.tensor_sub(den[:], S[:], a[:])
                bnum = small.tile([2, 1], f32, tag="bnum")
                nc.vector.tensor_mul(bnum[:], V[:], S[:])
                num = small.tile([2, 1], f32, tag="num")
                nc.vector.tensor_scalar_sub(num[:], bnum[:], CN)
                dinv = small.tile([2, 1], f32, tag="dinv")
                nc.vector.reciprocal(dinv[:], den[:])
                q = small.tile([2, 1], f32, tag="q")
                nc.vector.tensor_mul(q[:], num[:], dinv[:])
                Vn = small.tile([2, 1], f32, tag="V")
                nc.vector.tensor_sub(Vn[:], V[:], q[:])
                V = Vn

            # U = 1/(N*(V+E)), m = 1/MU - (N*V/MU)*U
            NV = small.tile([2, 1], f32, tag="NV")
            nc.vector.tensor_scalar_mul(NV[:], V[:], float(N))
            negNVmu = small.tile([2, 1], f32, tag="negNVmu")
            nc.vector.tensor_scalar_mul(negNVmu[:], NV[:], -1.0 / MU)
            nc.vector.tensor_scalar_add(tE[:], Epair[:], V[:])
            zN = scr.tile([2, N], f32, tag="zN", bufs=2)
            nc.vector.tensor_scalar_mul(zN[:], tE[:], float(N))
            Urow = small.tile([2, N], f32, tag="Urow", bufs=2)
            nc.vector.reciprocal_approx_accurate(Urow[:], zN[:], rscr[:])
            mrow = small.tile([2, N], f32, tag="mrow", bufs=2)
            nc.vector.tensor_scalar(mrow[:], Urow[:], negNVmu[:], 1.0 / MU,
                                    op0=A.mult, op1=A.add)

            # --- Pass C (same pair) ---
            upair = small.tile([1, N2], f32, tag="upair", bufs=2)
            mpair = small.tile([1, N2], f32, tag="mpair", bufs=2)
            for b2 in range(2):
                nc.gpsimd.dma_start(out=upair[0:1, b2 * N:(b2 + 1) * N],
                                    in_=Urow[b2:b2 + 1, :])
                nc.gpsimd.dma_start(out=mpair[0:1, b2 * N:(b2 + 1) * N],
                                    in_=mrow[b2:b2 + 1, :])

            ubc = ps1.tile([K, N2], f32, tag="ps1")
            nc.tensor.matmul(ubc[:], lhsT=ones_row[0:1, 0:K], rhs=upair[:],
                             start=True, stop=True)
            zt = scr.tile([K, N], f32, tag="zt", bufs=1)
            for b2 in range(2):
                nc.vector.affine_mul_reduce(
                    out=zt[:], accum_out=attr_all[:, 2 * p + b2:2 * p + b2 + 1],
                    in0=KKp[:, b2 * N:(b2 + 1) * N],
                    in1=ubc[:, b2 * N:(b2 + 1) * N], scale=1.0, bias=0.0)

            mbc = ps1.tile([128, N2], f32, tag="ps1")
            nc.tensor.matmul(mbc[:], lhsT=ones_row[:], rhs=mpair[:],
                             start=True, stop=True)
            mbc_sb = scr.tile([128, N2], f32, tag="mbcsb")
            nc.scalar.activation(mbc_sb[:], mbc[:], F.Copy)

            yscr = scr.tile([128, N], f32, tag="yscr")
            for b2 in range(2):
                ftc = featp.tile([128, CT, N], f32, tag="featC", bufs=3)
                nc.sync.dma_start(
                    out=ftc[:],
                    in_=feat_d[2 * p + b2].rearrange("(ct q) n -> q ct n", q=128))
                for ci in range(CT):
                    nc.vector._custom_dve(
                        MULMAX, out=yscr[:], in0=ftc[:, ci, :],
                        in1=mbc_sb[:, b2 * N:(b2 + 1) * N],
                        accum_out=gmp[:, ci, 2 * p + b2:2 * p + b2 + 1])

        # ---------- tail ----------
        ghi = singles.tile([128, CT, BPC], bf16)
        nc.scalar.activation(ghi[:], gmp[:], F.Copy)
        gdiff = scr.tile([128, CT, BPC], f32, tag="gdiff", bufs=1)
        nc.vector.tensor_sub(gdiff[:], gmp[:], ghi[:])
        glo = singles.tile([128, CT, BPC], bf16)
        nc.scalar.activation(glo[:], gdiff[:], F.Copy)
        xe_ps = ps1.tile([BPC, D], f32, tag="ps1")
        nc.tensor.matmul(xe_ps[:], lhsT=ones_row[0:1, 0:BPC], rhs=ebrow_sb[:],
                         start=True, stop=False)
        for ci in range(CT):
            with tc.tile_critical():
                nc.tensor.matmul(xe_ps[:], lhsT=ghi[:, ci, :], rhs=whi_sb[:, ci, :],
                                 start=False, stop=False)
                j2 = nc.tensor.matmul(xe_ps[:], lhsT=ghi[:, ci, :],
                                      rhs=wlo_sb[:, ci, :], start=False, stop=False)
                j2.ldweights = False
            nc.tensor.matmul(xe_ps[:], lhsT=glo[:, ci, :], rhs=whi_sb[:, ci, :],
                             start=False, stop=(ci == CT - 1))
        xe_sb = singles.tile([BPC, D], f32)
        nc.scalar.activation(xe_sb[:], xe_ps[:], F.Copy)

        # x_emb = l2n(xe)
        sqe = scr.tile([BPC, D], f32, tag="sqe", bufs=1)
        ssqe = small.tile([BPC, 1], f32, tag="ssqe")
        nc.vector.affine_mul_reduce(out=sqe[:], accum_out=ssqe[:], in0=xe_sb[:],
                                    in1=xe_sb[:], scale=1.0, bias=0.0)
        le = small.tile([BPC, 1], f32, tag="le")
        nc.scalar.activation(le[:], ssqe[:], F.Ln)
        einv = small.tile([BPC, 1], f32, tag="einv")
        nc.scalar.activation(einv[:], le[:], F.Exp, scale=-0.5)
        nc.vector.tensor_scalar_min(einv[:], einv[:], 1e12)
        emb_sb = singles.tile([BPC, D], f32)
        nc.vector.tensor_scalar_mul(emb_sb[:], xe_sb[:], einv[:])
        nc.sync.dma_start(out=emb_d, in_=emb_sb[:])

        # x_probs = xe @ cwT + cb
        xeT = singles.tile([128, DT, BPC], f32)
        for j in range(DT):
            tp = ps1.tile([128, BPC], f32, tag="ps1")
            nc.tensor.transpose(tp[:], xe_sb[:, j * 128:(j + 1) * 128],
                                ident[0:BPC, 0:BPC])
            nc.scalar.activation(xeT[:, j, :], tp[:], F.Copy)
        pr_ps = ps1.tile([BPC, NCLS], f32, tag="ps1")
        for j in range(DT):
            nc.tensor.matmul(pr_ps[:], lhsT=xeT[:, j, :], rhs=cwT_sb[:, j, :],
                             start=(j == 0), stop=False)
        nc.tensor.matmul(pr_ps[:], lhsT=ones_row[0:1, 0:BPC], rhs=cbrow_sb[:],
                         start=False, stop=True)
        pr_sb = singles.tile([BPC, NCLS], f32)
        nc.scalar.activation(pr_sb[:], pr_ps[:], F.Copy)
        nc.sync.dma_start(out=probs_d, in_=pr_sb[:])

        # x_attr: scale by 1/MU, transpose [K, BPC] -> [BPC, K]
        attr_s = scr.tile([K, BPC], f32, tag="attr_s", bufs=1)
        nc.vector.tensor_scalar_mul(attr_s[:], attr_all[:], 1.0 / MU)
        at_ps = ps1.tile([BPC, K], f32, tag="ps1")
        nc.tensor.transpose(at_ps[:], attr_s[:], ident[0:K, 0:K])
        at_sb = singles.tile([BPC, K], f32)
        nc.scalar.activation(at_sb[:], at_ps[:], F.Copy)
        nc.sync.dma_start(out=attr_d, in_=at_sb[:])

    nc.compile()
    return nc


def kernel(features, embed_w, embed_b, cls_w, cls_b, prototypes):
    from concourse.bass_utils import run_bass_kernel_spmd

    feats = np.ascontiguousarray(np.asarray(features, np.float32).reshape(B, C, N))
    ew = np.asarray(embed_w, np.float32)
    eb = np.asarray(embed_b, np.float32)
    cw = np.asarray(cls_w, np.float32)
    cb = np.asarray(cls_b, np.float32)
    pt = np.ascontiguousarray(np.asarray(prototypes, np.float32))

    import ml_dtypes
    bf = ml_dtypes.bfloat16
    ewT = np.ascontiguousarray(ew.T)                      # (C, D)
    ewThi = ewT.astype(bf)
    ewTlo = (ewT - ewThi.astype(np.float32)).astype(bf)
    fhi = feats.astype(bf)
    flo = (feats - fhi.astype(np.float32)).astype(bf)
    cwT = np.ascontiguousarray(cw.T)                      # (D, NCLS)
    ebrow = np.ascontiguousarray(eb.reshape(1, D))
    cbrow = np.ascontiguousarray(cb.reshape(1, NCLS))

    if "nc" not in _cache:
        _cache["nc"] = _build_nc()
    nc = _cache["nc"]

    in_maps = []
    for i in range(NCORES):
        in_maps.append({
            "features": np.ascontiguousarray(feats[i * BPC:(i + 1) * BPC]),
            "fhi": np.ascontiguousarray(fhi[i * BPC:(i + 1) * BPC]),
            "flo": np.ascontiguousarray(flo[i * BPC:(i + 1) * BPC]),
            "ewThi": ewThi, "ewTlo": ewTlo, "ebrow": ebrow,
            "cwT": cwT, "cbrow": cbrow, "prototypes": pt,
        })

    trace = bool(os.environ.get("KERNEL_TRACE"))
    res = run_bass_kernel_spmd(nc, in_maps, core_ids=list(range(NCORES)),
                               trace=trace)
    if trace and res.exec_time_ns is not None:
        print(f"HW exec time: {res.exec_time_ns} ns")
        _cache["exec_time_ns"] = res.exec_time_ns
        _cache["results_obj"] = res

    probs = np.concatenate([res.results[i]["x_probs"] for i in range(NCORES)], 0)
    emb = np.concatenate([res.results[i]["x_emb"] for i in range(NCORES)], 0)
    attr = np.concatenate([res.results[i]["x_attr"] for i in range(NCORES)], 0)
    return probs, emb, attr


# revision 36
# speedup vs baseline: 2.9797x; 2.9797x over previous
"""Trainium2 Bass kernel for the vq_codebook problem (nn_GSP_37890201485791).

Data-parallel over batch across 8 NeuronCores; params replicated.
Self-contained: shapes hardcoded, no sibling imports.
"""
import os
import numpy as np
from contextlib import ExitStack

B, C, H, W = 128, 2048, 14, 14
N = H * W               # 196
D, K, NCLS = 512, 64, 100
MU, EPS = 0.3, 0.1
NCORES = 8
BPC = B // NCORES       # 16 batches per core
NPAIR = BPC // 2        # 8 pairs per core
N2 = 2 * N              # 392
CT = C // 128           # 16 channel chunks
DT = D // 128           # 4 embed chunks
CN = float((1.0 - MU) * N)
NEWTON_ITERS = 8

_cache = {}


def _register_mul_max():
    """Author a fused (in0*in1, max-reduce) custom-DVE op at runtime.

    The ISA TensorTensorReduce opcode faults on this runtime; the
    custom-DVE table path (same one reciprocal_approx uses) works.
    """
    import concourse.dve_ops as dvo
    from concourse.dve_spec import Spec, Src0, Src1, MaxNeg, maxx, lower
    from concourse.dve_uop import DveOpSpec

    name = "ANT_MUL_MAX_REDUCE"
    if name in dvo._SUB_OPCODE_FOR_NAME:
        return next(op for op in dvo.OPS if op.name == name)
    def _ref(in0, in1, c0, c1, c2):
        b = (in0.astype(np.float32) * in1).astype(np.float32)
        return b, b.reshape(b.shape[0], -1).max(axis=-1, keepdims=True)

    spec = Spec(body=Src0 * Src1, accum=maxx, accum_init=MaxNeg, reference=_ref)
    opcode = dvo._CUSTOM_DVE_ROW_BASE + len(dvo.OPS)
    assert opcode < 0x20
    shas = {}
    for ver in ("v3", "v4"):
        u = lower(spec, ver=ver)
        shas[ver] = DveOpSpec(name=name, opcode=opcode, uops=u, rd1_en=True).sha(ver)
    op = dvo.DveOp(name, spec, subdim=False, uops_sha=shas)
    dvo.OPS.append(op)
    dvo._SUB_OPCODE_FOR_NAME[name] = opcode
    dvo.CUSTOM_DVE_SPECS[name] = spec
    return op


def _pin_act_tables():
    """Force every activation we use into one table set so the ACT engine
    loads its spline tables exactly once (Exp/Ln otherwise alternate sets)."""
    import concourse.bacc as bc
    import concourse.hw_specs as hws
    from concourse import mybir

    if getattr(bc, "_ant_act_pin", False):
        return
    F = mybir.ActivationFunctionType
    mine = {F.Copy, F.Identity, F.Exp, F.Ln, F.Square}
    keep = "natural_log_exp_and_others"
    orig = hws.get_activation_tables

    def patched(arch):
        real = orig(arch)
        return {name: (fns if name == keep else fns - mine)
                for name, fns in real.items()}

    bc.get_activation_tables = patched
    bc._ant_act_pin = True


def _build_nc():
    import concourse.tile as tile
    from concourse import bacc, mybir
    from concourse.masks import make_identity

    MULMAX = _register_mul_max()
    _pin_act_tables()

    f32 = mybir.dt.float32
    A = mybir.AluOpType
    F = mybir.ActivationFunctionType
    AX = mybir.AxisListType

    nc = bacc.Bacc("TRN2", target_bir_lowering=False, debug=False,
                   num_devices=NCORES)

    bf16 = mybir.dt.bfloat16
    feat_d = nc.dram_tensor("features", [BPC, C, N], f32, kind="ExternalInput").ap()
    fhi_d = nc.dram_tensor("fhi", [BPC, C, N], bf16, kind="ExternalInput").ap()
    flo_d = nc.dram_tensor("flo", [BPC, C, N], bf16, kind="ExternalInput").ap()
    whi_d = nc.dram_tensor("ewThi", [C, D], bf16, kind="ExternalInput").ap()
    wlo_d = nc.dram_tensor("ewTlo", [C, D], bf16, kind="ExternalInput").ap()
    ebrow_d = nc.dram_tensor("ebrow", [1, D], f32, kind="ExternalInput").ap()
    cwT_d = nc.dram_tensor("cwT", [D, NCLS], f32, kind="ExternalInput").ap()
    cbrow_d = nc.dram_tensor("cbrow", [1, NCLS], f32, kind="ExternalInput").ap()
    proto_d = nc.dram_tensor("prototypes", [K, D], f32, kind="ExternalInput").ap()

    probs_d = nc.dram_tensor("x_probs", [BPC, NCLS], f32, kind="ExternalOutput").ap()
    emb_d = nc.dram_tensor("x_emb", [BPC, D], f32, kind="ExternalOutput").ap()
    attr_d = nc.dram_tensor("x_attr", [BPC, K], f32, kind="ExternalOutput").ap()

    with tile.TileContext(nc) as tc, ExitStack() as ctx:
        singles = ctx.enter_context(tc.tile_pool(name="singles", bufs=1))
        featp = ctx.enter_context(tc.tile_pool(name="featp", bufs=3))
        bowp = ctx.enter_context(tc.tile_pool(name="bowp", bufs=5))
        small = ctx.enter_context(tc.tile_pool(name="small", bufs=4))
        scr = ctx.enter_context(tc.tile_pool(name="scr", bufs=2))
        ps_bow = ctx.enter_context(tc.tile_pool(name="ps_bow", bufs=4, space="PSUM"))
        ps1 = ctx.enter_context(tc.tile_pool(name="ps1", bufs=4, space="PSUM"))

        # ---------- constants / params ----------
        whi_sb = singles.tile([128, CT, D], bf16)
        wlo_sb = singles.tile([128, CT, D], bf16)
        for ci in range(CT):
            nc.sync.dma_start(out=whi_sb[:, ci, :], in_=whi_d[ci * 128:(ci + 1) * 128, :])
            nc.sync.dma_start(out=wlo_sb[:, ci, :], in_=wlo_d[ci * 128:(ci + 1) * 128, :])
        cwT_sb = singles.tile([128, DT, NCLS], f32)
        for j in range(DT):
            nc.sync.dma_start(out=cwT_sb[:, j, :], in_=cwT_d[j * 128:(j + 1) * 128, :])
        ebrow_sb = singles.tile([1, D], f32)
        nc.sync.dma_start(out=ebrow_sb[:], in_=ebrow_d)
        cbrow_sb = singles.tile([1, NCLS], f32)
        nc.sync.dma_start(out=cbrow_sb[:], in_=cbrow_d)

        ident = singles.tile([128, 128], f32)
        make_identity(nc, ident[:])
        ones_col = singles.tile([128, 1], f32)
        nc.vector.memset(ones_col[:], 1.0)
        ones_row = singles.tile([1, 128], f32)
        nc.vector.memset(ones_row[:], 1.0)
        bias_m20 = singles.tile([128, 1], f32)
        nc.vector.memset(bias_m20[:], -20.0)
        ones392 = singles.tile([1, N2], f32)
        nc.vector.memset(ones392[:], 1.0)

        # prototypes -> normalized, transposed  phatT [128, DT, K]
        proto_sb = singles.tile([K, D], f32)
        nc.sync.dma_start(out=proto_sb[:], in_=proto_d)
        psq = scr.tile([K, D], f32, tag="psq", bufs=1)
        pssq = small.tile([K, 1], f32, tag="pssq")
        nc.vector.affine_mul_reduce(out=psq[:], accum_out=pssq[:], in0=proto_sb[:],
                                    in1=proto_sb[:], scale=1.0, bias=0.0)
        plog = small.tile([K, 1], f32, tag="plog")
        nc.scalar.activation(plog[:], pssq[:], F.Ln)
        pinv = small.tile([K, 1], f32, tag="pinv")
        nc.scalar.activation(pinv[:], plog[:], F.Exp, scale=-0.5)
        nc.vector.tensor_scalar_min(pinv[:], pinv[:], 1e12)
        phat = singles.tile([K, D], f32)
        nc.vector.tensor_scalar_mul(phat[:], proto_sb[:], pinv[:])
        phatT = singles.tile([128, DT, K], f32)
        for j in range(DT):
            tp = ps1.tile([128, K], f32, tag="ps1")
            nc.tensor.transpose(tp[:], phat[:, j * 128:(j + 1) * 128],
                                ident[0:K, 0:K])
            nc.scalar.activation(phatT[:, j, :], tp[:], F.Copy)

        # persistent state
        gmp = singles.tile([128, CT, BPC], f32)
        attr_all = singles.tile([K, BPC], f32)

        # ---------- Pass A ----------
        for p in range(NPAIR):
            fthi = featp.tile([128, CT, 2, N], bf16, tag="fthi", bufs=2)
            ftlo = featp.tile([128, CT, 2, N], bf16, tag="ftlo", bufs=2)
            for b2 in range(2):
                nc.sync.dma_start(
                    out=fthi[:, :, b2, :],
                    in_=fhi_d[2 * p + b2].rearrange("(ct q) n -> q ct n", q=128))
                nc.sync.dma_start(
                    out=ftlo[:, :, b2, :],
                    in_=flo_d[2 * p + b2].rearrange("(ct q) n -> q ct n", q=128))

            bows = []
            for d in range(DT):
                bps = ps_bow.tile([128, N2], f32, tag="bowps")
                # bias: eb[d-chunk] outer-product ones, then accumulate conv
                nc.tensor.matmul(bps[:], lhsT=ebrow_sb[0:1, d * 128:(d + 1) * 128],
                                 rhs=ones392[:], start=True, stop=False)
                for ci in range(CT):
                    ds_ = slice(d * 128, (d + 1) * 128)
                    nc.tensor.matmul(bps[:], lhsT=whi_sb[:, ci, ds_],
                                     rhs=fthi[:, ci, :, :], start=False,
                                     stop=False)
                    i2 = nc.tensor.matmul(bps[:], lhsT=whi_sb[:, ci, ds_],
                                          rhs=ftlo[:, ci, :, :], start=False,
                                          stop=False)
                    i2.ldweights = False
                    nc.tensor.matmul(bps[:], lhsT=wlo_sb[:, ci, ds_],
                                     rhs=fthi[:, ci, :, :], start=False,
                                     stop=(ci == CT - 1))
                bow_sb = bowp.tile([128, N2], f32, tag="bow")
                nc.scalar.activation(bow_sb[:], bps[:], F.Copy)
                bows.append(bow_sb)

            gps = ps1.tile([K, N2], f32, tag="ps1")
            for d in range(DT):
                nc.tensor.matmul(gps[:], lhsT=phatT[:, d, :], rhs=bows[d][:],
                                 start=(d == 0), stop=(d == DT - 1))
            sps = ps1.tile([1, N2], f32, tag="ps1")
            for d in range(DT):
                sq_sb = scr.tile([128, N2], f32, tag="sq")
                nc.vector.tensor_mul(sq_sb[:], bows[d][:], bows[d][:])
                nc.tensor.matmul(sps[:], lhsT=ones_col[:], rhs=sq_sb[:],
                                 start=(d == 0), stop=(d == DT - 1))
            lssq = small.tile([1, N2], f32, tag="lssq", bufs=2)
            nc.scalar.activation(lssq[:], sps[:], F.Ln)
            rinv = small.tile([1, N2], f32, tag="rinv", bufs=2)
            nc.scalar.activation(rinv[:], lssq[:], F.Exp, scale=-0.5)
            rbc = ps1.tile([K, N2], f32, tag="ps1")
            nc.tensor.matmul(rbc[:], lhsT=ones_row[0:1, 0:K], rhs=rinv[:],
                             start=True, stop=True)
            rbc_sb = scr.tile([K, N2], f32, tag="rbcsb", bufs=1)
            nc.scalar.activation(rbc_sb[:], rbc[:], F.Copy)
            tcos = scr.tile([K, N2], f32, tag="tcos", bufs=1)
            nc.vector.tensor_mul(tcos[:], gps[:], rbc_sb[:])
            kk0 = scr.tile([K, N2], f32, tag="kk0", bufs=1)
            nc.scalar.activation(kk0[:], tcos[:], F.Exp, scale=20.0,
                                 bias=bias_m20[0:K, :])
            KKp = scr.tile([K, N2], f32, tag="KKp", bufs=2)
            nc.vector.tensor_scalar_min(KKp[:], kk0[:], 1.0)
            eps_ = ps1.tile([1, N2], f32, tag="ps1")
            nc.tensor.matmul(eps_[:], lhsT=ones_col[0:K, :], rhs=KKp[:],
                             start=True, stop=True)
            eps_sb = small.tile([1, N2], f32, tag="eps_sb", bufs=2)
            nc.scalar.activation(eps_sb[:], eps_[:], F.Copy)
            Epair = small.tile([2, N], f32, tag="Epair", bufs=2)
            for b2 in range(2):
                nc.gpsimd.dma_start(out=Epair[b2:b2 + 1, :],
                                    in_=eps_sb[0:1, b2 * N:(b2 + 1) * N])

            # --- Sinkhorn: Newton from below, V0 = 0 (per pair, 2 rows) ---
            V = small.tile([2, 1], f32, tag="V")
            nc.vector.memset(V[:], 0.0)
            tE = scr.tile([2, N], f32, tag="tE", bufs=2)
            rec = scr.tile([2, N], f32, tag="rec", bufs=2)
            rscr = scr.tile([2, N], f32, tag="rscr", bufs=2)
            sqd = scr.tile([2, N], f32, tag="sqd", bufs=2)
            for it in range(NEWTON_ITERS):
                nc.vector.tensor_scalar_add(tE[:], Epair[:], V[:])
                nc.vector.reciprocal_approx_accurate(rec[:], tE[:], rscr[:])
                S = small.tile([2, 1], f32, tag="S")
                nc.vector.tensor_reduce(S[:], rec[:], axis=AX.X, op=A.add)
                S2 = small.tile([2, 1], f32, tag="S2")
                nc.vector.affine_mul_reduce(out=sqd[:], accum_out=S2[:], in0=rec[:],
                                            in1=rec[:], scale=1.0, bias=0.0)
                a = small.tile([2, 1], f32, tag="a")
                nc.vector.tensor_mul(a[:], V[:], S2[:])
                den = small.tile([2, 1], f32, tag="den")
                nc.vector.tensor_sub(den[:], S[:], a[:])
                bnum = small.tile([2, 1], f32, tag="bnum")
                nc.vector.tensor_mul(bnum[:], V[:], S[:])
                num = small.tile([2, 1], f32, tag="num")
                nc.vector.tensor_scalar_sub(num[:], bnum[:], CN)
                dinv = small.tile([2, 1], f32, tag="dinv")
                nc.vector.reciprocal(dinv[:], den[:])
                q = small.tile([2, 1], f32, tag="q")
                nc.vector.tensor_mul(q[:], num[:], dinv[:])
                Vn = small.tile([2, 1], f32, tag="V")
                nc.vector.tensor_sub(Vn[:], V[:], q[:])
                V = Vn

            # U = 1/(N*(V+E)), m = 1/MU - (N*V/MU)*U
            NV = small.tile([2, 1], f32, tag="NV")
            nc.vector.tensor_scalar_mul(NV[:], V[:], float(N))
            negNVmu = small.tile([2, 1], f32, tag="negNVmu")
            nc.vector.tensor_scalar_mul(negNVmu[:], NV[:], -1.0 / MU)
            nc.vector.tensor_scalar_add(tE[:], Epair[:], V[:])
            zN = scr.tile([2, N], f32, tag="zN", bufs=2)
            nc.vector.tensor_scalar_mul(zN[:], tE[:], float(N))
            Urow = small.tile([2, N], f32, tag="Urow", bufs=2)
            nc.vector.reciprocal_approx_accurate(Urow[:], zN[:], rscr[:])
            mrow = small.tile([2, N], f32, tag="mrow", bufs=2)
            nc.vector.tensor_scalar(mrow[:], Urow[:], negNVmu[:], 1.0 / MU,
                                    op0=A.mult, op1=A.add)

            # --- Pass C (same pair) ---
            upair = small.tile([1, N2], f32, tag="upair", bufs=2)
            mpair = small.tile([1, N2], f32, tag="mpair", bufs=2)
            for b2 in range(2):
                nc.gpsimd.dma_start(out=upair[0:1, b2 * N:(b2 + 1) * N],
                                    in_=Urow[b2:b2 + 1, :])
                nc.gpsimd.dma_start(out=mpair[0:1, b2 * N:(b2 + 1) * N],
                                    in_=mrow[b2:b2 + 1, :])

            ubc = ps1.tile([K, N2], f32, tag="ps1")
            nc.tensor.matmul(ubc[:], lhsT=ones_row[0:1, 0:K], rhs=upair[:],
                             start=True, stop=True)
            zt = scr.tile([K, N], f32, tag="zt", bufs=1)
            for b2 in range(2):
                nc.vector.affine_mul_reduce(
                    out=zt[:], accum_out=attr_all[:, 2 * p + b2:2 * p + b2 + 1],
                    in0=KKp[:, b2 * N:(b2 + 1) * N],
                    in1=ubc[:, b2 * N:(b2 + 1) * N], scale=1.0, bias=0.0)

            mbc = ps1.tile([128, N2], f32, tag="ps1")
            nc.tensor.matmul(mbc[:], lhsT=ones_row[:], rhs=mpair[:],
                             start=True, stop=True)
            mbc_sb = scr.tile([128, N2], f32, tag="mbcsb")
            nc.scalar.activation(mbc_sb[:], mbc[:], F.Copy)

            yscr = scr.tile([128, N], f32, tag="yscr")
            for b2 in range(2):
                ftc = featp.tile([128, CT, N], f32, tag="featC", bufs=3)
                nc.sync.dma_start(
                    out=ftc[:],
                    in_=feat_d[2 * p + b2].rearrange("(ct q) n -> q ct n", q=128))
                for ci in range(CT):
                    nc.vector._custom_dve(
                        MULMAX, out=yscr[:], in0=ftc[:, ci, :],
                        in1=mbc_sb[:, b2 * N:(b2 + 1) * N],
                        accum_out=gmp[:, ci, 2 * p + b2:2 * p + b2 + 1])

        # ---------- tail ----------
        ghi = singles.tile([128, CT, BPC], bf16)
        nc.scalar.activation(ghi[:], gmp[:], F.Copy)
        gdiff = scr.tile([128, CT, BPC], f32, tag="gdiff", bufs=1)
        nc.vector.tensor_sub(gdiff[:], gmp[:], ghi[:])
        glo = singles.tile([128, CT, BPC], bf16)
        nc.scalar.activation(glo[:], gdiff[:], F.Copy)
        xe_ps = ps1.tile([BPC, D], f32, tag="ps1")
        nc.tensor.matmul(xe_ps[:], lhsT=ones_row[0:1, 0:BPC], rhs=ebrow_sb[:],
                         start=True, stop=False)
        for ci in range(CT):
            nc.tensor.matmul(xe_ps[:], lhsT=ghi[:, ci, :], rhs=whi_sb[:, ci, :],
                             start=False, stop=False)
            j2 = nc.tensor.matmul(xe_ps[:], lhsT=ghi[:, ci, :],
                                  rhs=wlo_sb[:, ci, :], start=False, stop=False)
            j2.ldweights = False
            nc.tensor.matmul(xe_ps[:], lhsT=glo[:, ci, :], rhs=whi_sb[:, ci, :],
                             start=False, stop=(ci == CT - 1))
        xe_sb = singles.tile([BPC, D], f32)
        nc.scalar.activation(xe_sb[:], xe_ps[:], F.Copy)

        # x_emb = l2n(xe)
        sqe = scr.tile([BPC, D], f32, tag="sqe", bufs=1)
        ssqe = small.tile([BPC, 1], f32, tag="ssqe")
        nc.vector.affine_mul_reduce(out=sqe[:], accum_out=ssqe[:], in0=xe_sb[:],
                                    in1=xe_sb[:], scale=1.0, bias=0.0)
        le = small.tile([BPC, 1], f32, tag="le")
        nc.scalar.activation(le[:], ssqe[:], F.Ln)
        einv = small.tile([BPC, 1], f32, tag="einv")
        nc.scalar.activation(einv[:], le[:], F.Exp, scale=-0.5)
        nc.vector.tensor_scalar_min(einv[:], einv[:], 1e12)
        emb_sb = singles.tile([BPC, D], f32)
        nc.vector.tensor_scalar_mul(emb_sb[:], xe_sb[:], einv[:])
        nc.sync.dma_start(out=emb_d, in_=emb_sb[:])

        # x_probs = xe @ cwT + cb
        xeT = singles.tile([128, DT, BPC], f32)
        for j in range(DT):
            tp = ps1.tile([128, BPC], f32, tag="ps1")
            nc.tensor.transpose(tp[:], xe_sb[:, j * 128:(j + 1) * 128],
                                ident[0:BPC, 0:BPC])
            nc.scalar.activation(xeT[:, j, :], tp[:], F.Copy)
        pr_ps = ps1.tile([BPC, NCLS], f32, tag="ps1")
        for j in range(DT):
            nc.tensor.matmul(pr_ps[:], lhsT=xeT[:, j, :], rhs=cwT_sb[:, j, :],
                             start=(j == 0), stop=False)
        nc.tensor.matmul(pr_ps[:], lhsT=ones_row[0:1, 0:BPC], rhs=cbrow_sb[:],
                         start=False, stop=True)
        pr_sb = singles.tile([BPC, NCLS], f32)
        nc.scalar.activation(pr_sb[:], pr_ps[:], F.Copy)
        nc.sync.dma_start(out=probs_d, in_=pr_sb[:])

        # x_attr: scale by 1/MU, transpose [K, BPC] -> [BPC, K]
        attr_s = scr.tile([K, BPC], f32, tag="attr_s", bufs=1)
        nc.vector.tensor_scalar_mul(attr_s[:], attr_all[:], 1.0 / MU)
        at_ps = ps1.tile([BPC, K], f32, tag="ps1")
        nc.tensor.transpose(at_ps[:], attr_s[:], ident[0:K, 0:K])
        at_sb = singles.tile([BPC, K], f32)
        nc.scalar.activation(at_sb[:], at_ps[:], F.Copy)
        nc.sync.dma_start(out=attr_d, in_=at_sb[:])

    nc.compile()
    return nc


def kernel(features, embed_w, embed_b, cls_w, cls_b, prototypes):
    from concourse.bass_utils import run_bass_kernel_spmd

    feats = np.ascontiguousarray(np.asarray(features, np.float32).reshape(B, C, N))
    ew = np.asarray(embed_w, np.float32)
    eb = np.asarray(embed_b, np.float32)
    cw = np.asarray(cls_w, np.float32)
    cb = np.asarray(cls_b, np.float32)
    pt = np.ascontiguousarray(np.asarray(prototypes, np.float32))

    import ml_dtypes
    bf = ml_dtypes.bfloat16
    ewT = np.ascontiguousarray(ew.T)                      # (C, D)
    ewThi = ewT.astype(bf)
    ewTlo = (ewT - ewThi.astype(np.float32)).astype(bf)
    fhi = feats.astype(bf)
    flo = (feats - fhi.astype(np.float32)).astype(bf)
    cwT = np.ascontiguousarray(cw.T)                      # (D, NCLS)
    ebrow = np.ascontiguousarray(eb.reshape(1, D))
    cbrow = np.ascontiguousarray(cb.reshape(1, NCLS))

    if "nc" not in _cache:
        _cache["nc"] = _build_nc()
    nc = _cache["nc"]

    in_maps = []
    for i in range(NCORES):
        in_maps.append({
            "features": np.ascontiguousarray(feats[i * BPC:(i + 1) * BPC]),
            "fhi": np.ascontiguousarray(fhi[i * BPC:(i + 1) * BPC]),
            "flo": np.ascontiguousarray(flo[i * BPC:(i + 1) * BPC]),
            "ewThi": ewThi, "ewTlo": ewTlo, "ebrow": ebrow,
            "cwT": cwT, "cbrow": cbrow, "prototypes": pt,
        })

    trace = bool(os.environ.get("KERNEL_TRACE"))
    res = run_bass_kernel_spmd(nc, in_maps, core_ids=list(range(NCORES)),
                               trace=trace)
    if trace and res.exec_time_ns is not None:
        print(f"HW exec time: {res.exec_time_ns} ns")
        _cache["exec_time_ns"] = res.exec_time_ns
        _cache["results_obj"] = res

    probs = np.concatenate([res.results[i]["x_probs"] for i in range(NCORES)], 0)
    emb = np.concatenate([res.results[i]["x_emb"] for i in range(NCORES)], 0)
    attr = np.concatenate([res.results[i]["x_attr"] for i in range(NCORES)], 0)
    return probs, emb, attr


# revision 37
# speedup vs baseline: 3.0680x; 1.0296x over previous
"""Trainium2 Bass kernel for the vq_codebook problem (nn_GSP_37890201485791).

Data-parallel over batch across 8 NeuronCores; params replicated.
Self-contained: shapes hardcoded, no sibling imports.
"""
import os
import numpy as np
from contextlib import ExitStack

B, C, H, W = 128, 2048, 14, 14
N = H * W               # 196
D, K, NCLS = 512, 64, 100
MU, EPS = 0.3, 0.1
NCORES = 8
BPC = B // NCORES       # 16 batches per core
NPAIR = BPC // 2        # 8 pairs per core
N2 = 2 * N              # 392
CT = C // 128           # 16 channel chunks
DT = D // 128           # 4 embed chunks
CN = float((1.0 - MU) * N)
NEWTON_ITERS = 7

_cache = {}


def _register_mul_max():
    """Author a fused (in0*in1, max-reduce) custom-DVE op at runtime.

    The ISA TensorTensorReduce opcode faults on this runtime; the
    custom-DVE table path (same one reciprocal_approx uses) works.
    """
    import concourse.dve_ops as dvo
    from concourse.dve_spec import Spec, Src0, Src1, MaxNeg, maxx, lower
    from concourse.dve_uop import DveOpSpec

    name = "ANT_MUL_MAX_REDUCE"
    if name in dvo._SUB_OPCODE_FOR_NAME:
        return next(op for op in dvo.OPS if op.name == name)
    def _ref(in0, in1, c0, c1, c2):
        b = (in0.astype(np.float32) * in1).astype(np.float32)
        return b, b.reshape(b.shape[0], -1).max(axis=-1, keepdims=True)

    spec = Spec(body=Src0 * Src1, accum=maxx, accum_init=MaxNeg, reference=_ref)
    opcode = dvo._CUSTOM_DVE_ROW_BASE + len(dvo.OPS)
    assert opcode < 0x20
    shas = {}
    for ver in ("v3", "v4"):
        u = lower(spec, ver=ver)
        shas[ver] = DveOpSpec(name=name, opcode=opcode, uops=u, rd1_en=True).sha(ver)
    op = dvo.DveOp(name, spec, subdim=False, uops_sha=shas)
    dvo.OPS.append(op)
    dvo._SUB_OPCODE_FOR_NAME[name] = opcode
    dvo.CUSTOM_DVE_SPECS[name] = spec
    return op


def _pin_act_tables():
    """Force every activation we use into one table set so the ACT engine
    loads its spline tables exactly once (Exp/Ln otherwise alternate sets)."""
    import concourse.bacc as bc
    import concourse.hw_specs as hws
    from concourse import mybir

    if getattr(bc, "_ant_act_pin", False):
        return
    F = mybir.ActivationFunctionType
    mine = {F.Copy, F.Identity, F.Exp, F.Ln, F.Square}
    keep = "natural_log_exp_and_others"
    orig = hws.get_activation_tables

    def patched(arch):
        real = orig(arch)
        return {name: (fns if name == keep else fns - mine)
                for name, fns in real.items()}

    bc.get_activation_tables = patched
    bc._ant_act_pin = True


def _build_nc():
    import concourse.tile as tile
    from concourse import bacc, mybir
    from concourse.masks import make_identity

    MULMAX = _register_mul_max()
    _pin_act_tables()

    f32 = mybir.dt.float32
    A = mybir.AluOpType
    F = mybir.ActivationFunctionType
    AX = mybir.AxisListType

    nc = bacc.Bacc("TRN2", target_bir_lowering=False, debug=False,
                   num_devices=NCORES)

    bf16 = mybir.dt.bfloat16
    feat_d = nc.dram_tensor("features", [BPC, C, N], f32, kind="ExternalInput").ap()
    fhi_d = nc.dram_tensor("fhi", [BPC, C, N], bf16, kind="ExternalInput").ap()
    flo_d = nc.dram_tensor("flo", [BPC, C, N], bf16, kind="ExternalInput").ap()
    whi_d = nc.dram_tensor("ewThi", [C, D], bf16, kind="ExternalInput").ap()
    wlo_d = nc.dram_tensor("ewTlo", [C, D], bf16, kind="ExternalInput").ap()
    ebrow_d = nc.dram_tensor("ebrow", [1, D], f32, kind="ExternalInput").ap()
    cwT_d = nc.dram_tensor("cwT", [D, NCLS], f32, kind="ExternalInput").ap()
    cbrow_d = nc.dram_tensor("cbrow", [1, NCLS], f32, kind="ExternalInput").ap()
    proto_d = nc.dram_tensor("prototypes", [K, D], f32, kind="ExternalInput").ap()

    probs_d = nc.dram_tensor("x_probs", [BPC, NCLS], f32, kind="ExternalOutput").ap()
    emb_d = nc.dram_tensor("x_emb", [BPC, D], f32, kind="ExternalOutput").ap()
    attr_d = nc.dram_tensor("x_attr", [BPC, K], f32, kind="ExternalOutput").ap()

    with tile.TileContext(nc) as tc, ExitStack() as ctx:
        singles = ctx.enter_context(tc.tile_pool(name="singles", bufs=1))
        featp = ctx.enter_context(tc.tile_pool(name="featp", bufs=3))
        bowp = ctx.enter_context(tc.tile_pool(name="bowp", bufs=5))
        small = ctx.enter_context(tc.tile_pool(name="small", bufs=4))
        scr = ctx.enter_context(tc.tile_pool(name="scr", bufs=2))
        ps_bow = ctx.enter_context(tc.tile_pool(name="ps_bow", bufs=4, space="PSUM"))
        ps1 = ctx.enter_context(tc.tile_pool(name="ps1", bufs=4, space="PSUM"))

        # ---------- constants / params ----------
        whi_sb = singles.tile([128, CT, D], bf16)
        wlo_sb = singles.tile([128, CT, D], bf16)
        for ci in range(CT):
            nc.sync.dma_start(out=whi_sb[:, ci, :], in_=whi_d[ci * 128:(ci + 1) * 128, :])
            nc.sync.dma_start(out=wlo_sb[:, ci, :], in_=wlo_d[ci * 128:(ci + 1) * 128, :])
        cwT_sb = singles.tile([128, DT, NCLS], f32)
        for j in range(DT):
            nc.sync.dma_start(out=cwT_sb[:, j, :], in_=cwT_d[j * 128:(j + 1) * 128, :])
        ebrow_sb = singles.tile([1, D], f32)
        nc.sync.dma_start(out=ebrow_sb[:], in_=ebrow_d)
        cbrow_sb = singles.tile([1, NCLS], f32)
        nc.sync.dma_start(out=cbrow_sb[:], in_=cbrow_d)

        ident = singles.tile([128, 128], f32)
        make_identity(nc, ident[:])
        ones_col = singles.tile([128, 1], f32)
        nc.vector.memset(ones_col[:], 1.0)
        ones_row = singles.tile([1, 128], f32)
        nc.vector.memset(ones_row[:], 1.0)
        bias_m20 = singles.tile([128, 1], f32)
        nc.vector.memset(bias_m20[:], -20.0)
        ones392 = singles.tile([1, N2], f32)
        nc.vector.memset(ones392[:], 1.0)

        # prototypes -> normalized, transposed  phatT [128, DT, K]
        proto_sb = singles.tile([K, D], f32)
        nc.sync.dma_start(out=proto_sb[:], in_=proto_d)
        psq = scr.tile([K, D], f32, tag="psq", bufs=1)
        pssq = small.tile([K, 1], f32, tag="pssq")
        nc.vector.affine_mul_reduce(out=psq[:], accum_out=pssq[:], in0=proto_sb[:],
                                    in1=proto_sb[:], scale=1.0, bias=0.0)
        plog = small.tile([K, 1], f32, tag="plog")
        nc.scalar.activation(plog[:], pssq[:], F.Ln)
        pinv = small.tile([K, 1], f32, tag="pinv")
        nc.scalar.activation(pinv[:], plog[:], F.Exp, scale=-0.5)
        nc.vector.tensor_scalar_min(pinv[:], pinv[:], 1e12)
        phat = singles.tile([K, D], f32)
        nc.vector.tensor_scalar_mul(phat[:], proto_sb[:], pinv[:])
        phatT = singles.tile([128, DT, K], f32)
        for j in range(DT):
            tp = ps1.tile([128, K], f32, tag="ps1")
            nc.tensor.transpose(tp[:], phat[:, j * 128:(j + 1) * 128],
                                ident[0:K, 0:K])
            nc.scalar.activation(phatT[:, j, :], tp[:], F.Copy)

        # persistent state
        gmp = singles.tile([128, CT, BPC], f32)
        attr_all = singles.tile([K, BPC], f32)

        # ---------- Pass A ----------
        for p in range(NPAIR):
            fthi = featp.tile([128, CT, 2, N], bf16, tag="fthi", bufs=2)
            ftlo = featp.tile([128, CT, 2, N], bf16, tag="ftlo", bufs=2)
            for b2 in range(2):
                nc.sync.dma_start(
                    out=fthi[:, :, b2, :],
                    in_=fhi_d[2 * p + b2].rearrange("(ct q) n -> q ct n", q=128))
                nc.sync.dma_start(
                    out=ftlo[:, :, b2, :],
                    in_=flo_d[2 * p + b2].rearrange("(ct q) n -> q ct n", q=128))

            bows = []
            for d in range(DT):
                bps = ps_bow.tile([128, N2], f32, tag="bowps")
                # bias: eb[d-chunk] outer-product ones, then accumulate conv
                nc.tensor.matmul(bps[:], lhsT=ebrow_sb[0:1, d * 128:(d + 1) * 128],
                                 rhs=ones392[:], start=True, stop=False)
                for ci in range(CT):
                    ds_ = slice(d * 128, (d + 1) * 128)
                    nc.tensor.matmul(bps[:], lhsT=whi_sb[:, ci, ds_],
                                     rhs=fthi[:, ci, :, :], start=False,
                                     stop=False)
                    i2 = nc.tensor.matmul(bps[:], lhsT=whi_sb[:, ci, ds_],
                                          rhs=ftlo[:, ci, :, :], start=False,
                                          stop=False)
                    i2.ldweights = False
                    nc.tensor.matmul(bps[:], lhsT=wlo_sb[:, ci, ds_],
                                     rhs=fthi[:, ci, :, :], start=False,
                                     stop=(ci == CT - 1))
                bow_sb = bowp.tile([128, N2], f32, tag="bow")
                nc.scalar.activation(bow_sb[:], bps[:], F.Copy)
                bows.append(bow_sb)

            gps = ps1.tile([K, N2], f32, tag="ps1")
            for d in range(DT):
                nc.tensor.matmul(gps[:], lhsT=phatT[:, d, :], rhs=bows[d][:],
                                 start=(d == 0), stop=(d == DT - 1))
            sps = ps1.tile([1, N2], f32, tag="ps1")
            for d in range(DT):
                sq_sb = scr.tile([128, N2], f32, tag="sq")
                nc.vector.tensor_mul(sq_sb[:], bows[d][:], bows[d][:])
                nc.tensor.matmul(sps[:], lhsT=ones_col[:], rhs=sq_sb[:],
                                 start=(d == 0), stop=(d == DT - 1))
            lssq = small.tile([1, N2], f32, tag="lssq", bufs=2)
            nc.scalar.activation(lssq[:], sps[:], F.Ln)
            rinv = small.tile([1, N2], f32, tag="rinv", bufs=2)
            nc.scalar.activation(rinv[:], lssq[:], F.Exp, scale=-0.5)
            rbc = ps1.tile([K, N2], f32, tag="ps1")
            nc.tensor.matmul(rbc[:], lhsT=ones_row[0:1, 0:K], rhs=rinv[:],
                             start=True, stop=True)
            rbc_sb = scr.tile([K, N2], f32, tag="rbcsb", bufs=1)
            nc.scalar.activation(rbc_sb[:], rbc[:], F.Copy)
            tcos = scr.tile([K, N2], f32, tag="tcos", bufs=1)
            nc.vector.tensor_mul(tcos[:], gps[:], rbc_sb[:])
            kk0 = scr.tile([K, N2], f32, tag="kk0", bufs=1)
            nc.scalar.activation(kk0[:], tcos[:], F.Exp, scale=20.0,
                                 bias=bias_m20[0:K, :])
            KKp = scr.tile([K, N2], f32, tag="KKp", bufs=2)
            nc.vector.tensor_scalar_min(KKp[:], kk0[:], 1.0)
            eps_ = ps1.tile([1, N2], f32, tag="ps1")
            nc.tensor.matmul(eps_[:], lhsT=ones_col[0:K, :], rhs=KKp[:],
                             start=True, stop=True)
            eps_sb = small.tile([1, N2], f32, tag="eps_sb", bufs=2)
            nc.scalar.activation(eps_sb[:], eps_[:], F.Copy)
            Epair = small.tile([2, N], f32, tag="Epair", bufs=2)
            for b2 in range(2):
                nc.gpsimd.dma_start(out=Epair[b2:b2 + 1, :],
                                    in_=eps_sb[0:1, b2 * N:(b2 + 1) * N])

            # --- Sinkhorn: Newton from below, V0 = 0 (per pair, 2 rows) ---
            V = small.tile([2, 1], f32, tag="V")
            nc.vector.memset(V[:], 0.0)
            tE = scr.tile([2, N], f32, tag="tE", bufs=2)
            rec = scr.tile([2, N], f32, tag="rec", bufs=2)
            rscr = scr.tile([2, N], f32, tag="rscr", bufs=2)
            sqd = scr.tile([2, N], f32, tag="sqd", bufs=2)
            for it in range(NEWTON_ITERS):
                nc.vector.tensor_scalar_add(tE[:], Epair[:], V[:])
                nc.vector.reciprocal_approx_accurate(rec[:], tE[:], rscr[:])
                S = small.tile([2, 1], f32, tag="S")
                nc.vector.tensor_reduce(S[:], rec[:], axis=AX.X, op=A.add)
                S2 = small.tile([2, 1], f32, tag="S2")
                nc.vector.affine_mul_reduce(out=sqd[:], accum_out=S2[:], in0=rec[:],
                                            in1=rec[:], scale=1.0, bias=0.0)
                a = small.tile([2, 1], f32, tag="a")
                nc.vector.tensor_mul(a[:], V[:], S2[:])
                den = small.tile([2, 1], f32, tag="den")
                nc.vector.tensor_sub(den[:], S[:], a[:])
                bnum = small.tile([2, 1], f32, tag="bnum")
                nc.vector.tensor_mul(bnum[:], V[:], S[:])
                num = small.tile([2, 1], f32, tag="num")
                nc.vector.tensor_scalar_sub(num[:], bnum[:], CN)
                dinv = small.tile([2, 1], f32, tag="dinv")
                nc.vector.reciprocal(dinv[:], den[:])
                q = small.tile([2, 1], f32, tag="q")
                nc.vector.tensor_mul(q[:], num[:], dinv[:])
                Vn = small.tile([2, 1], f32, tag="V")
                nc.vector.tensor_sub(Vn[:], V[:], q[:])
                V = Vn

            # U = 1/(N*(V+E)), m = 1/MU - (N*V/MU)*U
            NV = small.tile([2, 1], f32, tag="NV")
            nc.vector.tensor_scalar_mul(NV[:], V[:], float(N))
            negNVmu = small.tile([2, 1], f32, tag="negNVmu")
            nc.vector.tensor_scalar_mul(negNVmu[:], NV[:], -1.0 / MU)
            nc.vector.tensor_scalar_add(tE[:], Epair[:], V[:])
            zN = scr.tile([2, N], f32, tag="zN", bufs=2)
            nc.vector.tensor_scalar_mul(zN[:], tE[:], float(N))
            Urow = small.tile([2, N], f32, tag="Urow", bufs=2)
            nc.vector.reciprocal_approx_accurate(Urow[:], zN[:], rscr[:])
            mrow = small.tile([2, N], f32, tag="mrow", bufs=2)
            nc.vector.tensor_scalar(mrow[:], Urow[:], negNVmu[:], 1.0 / MU,
                                    op0=A.mult, op1=A.add)

            # --- Pass C (same pair) ---
            upair = small.tile([1, N2], f32, tag="upair", bufs=2)
            mpair = small.tile([1, N2], f32, tag="mpair", bufs=2)
            for b2 in range(2):
                nc.gpsimd.dma_start(out=upair[0:1, b2 * N:(b2 + 1) * N],
                                    in_=Urow[b2:b2 + 1, :])
                nc.gpsimd.dma_start(out=mpair[0:1, b2 * N:(b2 + 1) * N],
                                    in_=mrow[b2:b2 + 1, :])

            ubc = ps1.tile([K, N2], f32, tag="ps1")
            nc.tensor.matmul(ubc[:], lhsT=ones_row[0:1, 0:K], rhs=upair[:],
                             start=True, stop=True)
            zt = scr.tile([K, N], f32, tag="zt", bufs=1)
            for b2 in range(2):
                nc.vector.affine_mul_reduce(
                    out=zt[:], accum_out=attr_all[:, 2 * p + b2:2 * p + b2 + 1],
                    in0=KKp[:, b2 * N:(b2 + 1) * N],
                    in1=ubc[:, b2 * N:(b2 + 1) * N], scale=1.0, bias=0.0)

            mbc = ps1.tile([128, N2], f32, tag="ps1")
            nc.tensor.matmul(mbc[:], lhsT=ones_row[:], rhs=mpair[:],
                             start=True, stop=True)
            mbc_sb = scr.tile([128, N2], f32, tag="mbcsb")
            nc.scalar.activation(mbc_sb[:], mbc[:], F.Copy)

            yscr = scr.tile([128, N], f32, tag="yscr")
            for b2 in range(2):
                ftc = featp.tile([128, CT, N], f32, tag="featC", bufs=3)
                nc.sync.dma_start(
                    out=ftc[:],
                    in_=feat_d[2 * p + b2].rearrange("(ct q) n -> q ct n", q=128))
                for ci in range(CT):
                    nc.vector._custom_dve(
                        MULMAX, out=yscr[:], in0=ftc[:, ci, :],
                        in1=mbc_sb[:, b2 * N:(b2 + 1) * N],
                        accum_out=gmp[:, ci, 2 * p + b2:2 * p + b2 + 1])

        # ---------- tail ----------
        ghi = singles.tile([128, CT, BPC], bf16)
        nc.scalar.activation(ghi[:], gmp[:], F.Copy)
        gdiff = scr.tile([128, CT, BPC], f32, tag="gdiff", bufs=1)
        nc.vector.tensor_sub(gdiff[:], gmp[:], ghi[:])
        glo = singles.tile([128, CT, BPC], bf16)
        nc.scalar.activation(glo[:], gdiff[:], F.Copy)
        xe_ps = ps1.tile([BPC, D], f32, tag="ps1")
        nc.tensor.matmul(xe_ps[:], lhsT=ones_row[0:1, 0:BPC], rhs=ebrow_sb[:],
                         start=True, stop=False)
        for ci in range(CT):
            nc.tensor.matmul(xe_ps[:], lhsT=ghi[:, ci, :], rhs=whi_sb[:, ci, :],
                             start=False, stop=False)
            j2 = nc.tensor.matmul(xe_ps[:], lhsT=ghi[:, ci, :],
                                  rhs=wlo_sb[:, ci, :], start=False, stop=False)
            j2.ldweights = False
            nc.tensor.matmul(xe_ps[:], lhsT=glo[:, ci, :], rhs=whi_sb[:, ci, :],
                             start=False, stop=(ci == CT - 1))
        xe_sb = singles.tile([BPC, D], f32)
        nc.scalar.activation(xe_sb[:], xe_ps[:], F.Copy)

        # x_emb = l2n(xe)
        sqe = scr.tile([BPC, D], f32, tag="sqe", bufs=1)
        ssqe = small.tile([BPC, 1], f32, tag="ssqe")
        nc.vector.affine_mul_reduce(out=sqe[:], accum_out=ssqe[:], in0=xe_sb[:],
                                    in1=xe_sb[:], scale=1.0, bias=0.0)
        le = small.tile([BPC, 1], f32, tag="le")
        nc.scalar.activation(le[:], ssqe[:], F.Ln)
        einv = small.tile([BPC, 1], f32, tag="einv")
        nc.scalar.activation(einv[:], le[:], F.Exp, scale=-0.5)
        nc.vector.tensor_scalar_min(einv[:], einv[:], 1e12)
        emb_sb = singles.tile([BPC, D], f32)
        nc.vector.tensor_scalar_mul(emb_sb[:], xe_sb[:], einv[:])
        nc.sync.dma_start(out=emb_d, in_=emb_sb[:])

        # x_probs = xe @ cwT + cb
        xeT = singles.tile([128, DT, BPC], f32)
        for j in range(DT):
            tp = ps1.tile([128, BPC], f32, tag="ps1")
            nc.tensor.transpose(tp[:], xe_sb[:, j * 128:(j + 1) * 128],
                                ident[0:BPC, 0:BPC])
            nc.scalar.activation(xeT[:, j, :], tp[:], F.Copy)
        pr_ps = ps1.tile([BPC, NCLS], f32, tag="ps1")
        for j in range(DT):
            nc.tensor.matmul(pr_ps[:], lhsT=xeT[:, j, :], rhs=cwT_sb[:, j, :],
                             start=(j == 0), stop=False)
        nc.tensor.matmul(pr_ps[:], lhsT=ones_row[0:1, 0:BPC], rhs=cbrow_sb[:],
                         start=False, stop=True)
        pr_sb = singles.tile([BPC, NCLS], f32)
        nc.scalar.activation(pr_sb[:], pr_ps[:], F.Copy)
        nc.sync.dma_start(out=probs_d, in_=pr_sb[:])

        # x_attr: scale by 1/MU, transpose [K, BPC] -> [BPC, K]
        attr_s = scr.tile([K, BPC], f32, tag="attr_s", bufs=1)
        nc.vector.tensor_scalar_mul(attr_s[:], attr_all[:], 1.0 / MU)
        at_ps = ps1.tile([BPC, K], f32, tag="ps1")
        nc.tensor.transpose(at_ps[:], attr_s[:], ident[0:K, 0:K])
        at_sb = singles.tile([BPC, K], f32)
        nc.scalar.activation(at_sb[:], at_ps[:], F.Copy)
        nc.sync.dma_start(out=attr_d, in_=at_sb[:])

    nc.compile()
    return nc


def kernel(features, embed_w, embed_b, cls_w, cls_b, prototypes):
    from concourse.bass_utils import run_bass_kernel_spmd

    feats = np.ascontiguousarray(np.asarray(features, np.float32).reshape(B, C, N))
    ew = np.asarray(embed_w, np.float32)
    eb = np.asarray(embed_b, np.float32)
    cw = np.asarray(cls_w, np.float32)
    cb = np.asarray(cls_b, np.float32)
    pt = np.ascontiguousarray(np.asarray(prototypes, np.float32))

    import ml_dtypes
    bf = ml_dtypes.bfloat16
    ewT = np.ascontiguousarray(ew.T)                      # (C, D)
    ewThi = ewT.astype(bf)
    ewTlo = (ewT - ewThi.astype(np.float32)).astype(bf)
    fhi = feats.astype(bf)
    flo = (feats - fhi.astype(np.float32)).astype(bf)
    cwT = np.ascontiguousarray(cw.T)                      # (D, NCLS)
    ebrow = np.ascontiguousarray(eb.reshape(1, D))
    cbrow = np.ascontiguousarray(cb.reshape(1, NCLS))

    if "nc" not in _cache:
        _cache["nc"] = _build_nc()
    nc = _cache["nc"]

    in_maps = []
    for i in range(NCORES):
        in_maps.append({
            "features": np.ascontiguousarray(feats[i * BPC:(i + 1) * BPC]),
            "fhi": np.ascontiguousarray(fhi[i * BPC:(i + 1) * BPC]),
            "flo": np.ascontiguousarray(flo[i * BPC:(i + 1) * BPC]),
            "ewThi": ewThi, "ewTlo": ewTlo, "ebrow": ebrow,
            "cwT": cwT, "cbrow": cbrow, "prototypes": pt,
        })

    trace = bool(os.environ.get("KERNEL_TRACE"))
    res = run_bass_kernel_spmd(nc, in_maps, core_ids=list(range(NCORES)),
                               trace=trace)
    if trace and res.exec_time_ns is not None:
        print(f"HW exec time: {res.exec_time_ns} ns")
        _cache["exec_time_ns"] = res.exec_time_ns
        _cache["results_obj"] = res

    probs = np.concatenate([res.results[i]["x_probs"] for i in range(NCORES)], 0)
    emb = np.concatenate([res.results[i]["x_emb"] for i in range(NCORES)], 0)
    attr = np.concatenate([res.results[i]["x_attr"] for i in range(NCORES)], 0)
    return probs, emb, attr


# revision 38
# speedup vs baseline: 3.1032x; 1.0115x over previous
"""Trainium2 Bass kernel for the vq_codebook problem (nn_GSP_37890201485791).

Data-parallel over batch across 8 NeuronCores; params replicated.
Self-contained: shapes hardcoded, no sibling imports.
"""
import os
import numpy as np
from contextlib import ExitStack

B, C, H, W = 128, 2048, 14, 14
N = H * W               # 196
D, K, NCLS = 512, 64, 100
MU, EPS = 0.3, 0.1
NCORES = 8
BPC = B // NCORES       # 16 batches per core
NPAIR = BPC // 2        # 8 pairs per core
N2 = 2 * N              # 392
CT = C // 128           # 16 channel chunks
DT = D // 128           # 4 embed chunks
CN = float((1.0 - MU) * N)
NEWTON_ITERS = 6

_cache = {}


def _register_mul_max():
    """Author a fused (in0*in1, max-reduce) custom-DVE op at runtime.

    The ISA TensorTensorReduce opcode faults on this runtime; the
    custom-DVE table path (same one reciprocal_approx uses) works.
    """
    import concourse.dve_ops as dvo
    from concourse.dve_spec import Spec, Src0, Src1, MaxNeg, maxx, lower
    from concourse.dve_uop import DveOpSpec

    name = "ANT_MUL_MAX_REDUCE"
    if name in dvo._SUB_OPCODE_FOR_NAME:
        return next(op for op in dvo.OPS if op.name == name)
    def _ref(in0, in1, c0, c1, c2):
        b = (in0.astype(np.float32) * in1).astype(np.float32)
        return b, b.reshape(b.shape[0], -1).max(axis=-1, keepdims=True)

    spec = Spec(body=Src0 * Src1, accum=maxx, accum_init=MaxNeg, reference=_ref)
    opcode = dvo._CUSTOM_DVE_ROW_BASE + len(dvo.OPS)
    assert opcode < 0x20
    shas = {}
    for ver in ("v3", "v4"):
        u = lower(spec, ver=ver)
        shas[ver] = DveOpSpec(name=name, opcode=opcode, uops=u, rd1_en=True).sha(ver)
    op = dvo.DveOp(name, spec, subdim=False, uops_sha=shas)
    dvo.OPS.append(op)
    dvo._SUB_OPCODE_FOR_NAME[name] = opcode
    dvo.CUSTOM_DVE_SPECS[name] = spec
    return op


def _pin_act_tables():
    """Force every activation we use into one table set so the ACT engine
    loads its spline tables exactly once (Exp/Ln otherwise alternate sets)."""
    import concourse.bacc as bc
    import concourse.hw_specs as hws
    from concourse import mybir

    if getattr(bc, "_ant_act_pin", False):
        return
    F = mybir.ActivationFunctionType
    mine = {F.Copy, F.Identity, F.Exp, F.Ln, F.Square}
    keep = "natural_log_exp_and_others"
    orig = hws.get_activation_tables

    def patched(arch):
        real = orig(arch)
        return {name: (fns if name == keep else fns - mine)
                for name, fns in real.items()}

    bc.get_activation_tables = patched
    bc._ant_act_pin = True


def _build_nc():
    import concourse.tile as tile
    from concourse import bacc, mybir
    from concourse.masks import make_identity

    MULMAX = _register_mul_max()
    _pin_act_tables()

    f32 = mybir.dt.float32
    A = mybir.AluOpType
    F = mybir.ActivationFunctionType
    AX = mybir.AxisListType

    nc = bacc.Bacc("TRN2", target_bir_lowering=False, debug=False,
                   num_devices=NCORES)

    bf16 = mybir.dt.bfloat16
    feat_d = nc.dram_tensor("features", [BPC, C, N], f32, kind="ExternalInput").ap()
    fhi_d = nc.dram_tensor("fhi", [BPC, C, N], bf16, kind="ExternalInput").ap()
    flo_d = nc.dram_tensor("flo", [BPC, C, N], bf16, kind="ExternalInput").ap()
    whi_d = nc.dram_tensor("ewThi", [C, D], bf16, kind="ExternalInput").ap()
    wlo_d = nc.dram_tensor("ewTlo", [C, D], bf16, kind="ExternalInput").ap()
    ebrow_d = nc.dram_tensor("ebrow", [1, D], f32, kind="ExternalInput").ap()
    cwT_d = nc.dram_tensor("cwT", [D, NCLS], f32, kind="ExternalInput").ap()
    cbrow_d = nc.dram_tensor("cbrow", [1, NCLS], f32, kind="ExternalInput").ap()
    proto_d = nc.dram_tensor("prototypes", [K, D], f32, kind="ExternalInput").ap()

    probs_d = nc.dram_tensor("x_probs", [BPC, NCLS], f32, kind="ExternalOutput").ap()
    emb_d = nc.dram_tensor("x_emb", [BPC, D], f32, kind="ExternalOutput").ap()
    attr_d = nc.dram_tensor("x_attr", [BPC, K], f32, kind="ExternalOutput").ap()

    with tile.TileContext(nc) as tc, ExitStack() as ctx:
        singles = ctx.enter_context(tc.tile_pool(name="singles", bufs=1))
        featp = ctx.enter_context(tc.tile_pool(name="featp", bufs=3))
        bowp = ctx.enter_context(tc.tile_pool(name="bowp", bufs=5))
        small = ctx.enter_context(tc.tile_pool(name="small", bufs=4))
        scr = ctx.enter_context(tc.tile_pool(name="scr", bufs=2))
        ps_bow = ctx.enter_context(tc.tile_pool(name="ps_bow", bufs=4, space="PSUM"))
        ps1 = ctx.enter_context(tc.tile_pool(name="ps1", bufs=4, space="PSUM"))

        # ---------- constants / params ----------
        whi_sb = singles.tile([128, CT, D], bf16)
        wlo_sb = singles.tile([128, CT, D], bf16)
        for ci in range(CT):
            nc.sync.dma_start(out=whi_sb[:, ci, :], in_=whi_d[ci * 128:(ci + 1) * 128, :])
            nc.sync.dma_start(out=wlo_sb[:, ci, :], in_=wlo_d[ci * 128:(ci + 1) * 128, :])
        cwT_sb = singles.tile([128, DT, NCLS], f32)
        for j in range(DT):
            nc.sync.dma_start(out=cwT_sb[:, j, :], in_=cwT_d[j * 128:(j + 1) * 128, :])
        ebrow_sb = singles.tile([1, D], f32)
        nc.sync.dma_start(out=ebrow_sb[:], in_=ebrow_d)
        cbrow_sb = singles.tile([1, NCLS], f32)
        nc.sync.dma_start(out=cbrow_sb[:], in_=cbrow_d)

        ident = singles.tile([128, 128], f32)
        make_identity(nc, ident[:])
        ones_col = singles.tile([128, 1], f32)
        nc.vector.memset(ones_col[:], 1.0)
        ones_row = singles.tile([1, 128], f32)
        nc.vector.memset(ones_row[:], 1.0)
        bias_m20 = singles.tile([128, 1], f32)
        nc.vector.memset(bias_m20[:], -20.0)
        ones392 = singles.tile([1, N2], f32)
        nc.vector.memset(ones392[:], 1.0)

        # prototypes -> normalized, transposed  phatT [128, DT, K]
        proto_sb = singles.tile([K, D], f32)
        nc.sync.dma_start(out=proto_sb[:], in_=proto_d)
        psq = scr.tile([K, D], f32, tag="psq", bufs=1)
        pssq = small.tile([K, 1], f32, tag="pssq")
        nc.vector.affine_mul_reduce(out=psq[:], accum_out=pssq[:], in0=proto_sb[:],
                                    in1=proto_sb[:], scale=1.0, bias=0.0)
        plog = small.tile([K, 1], f32, tag="plog")
        nc.scalar.activation(plog[:], pssq[:], F.Ln)
        pinv = small.tile([K, 1], f32, tag="pinv")
        nc.scalar.activation(pinv[:], plog[:], F.Exp, scale=-0.5)
        nc.vector.tensor_scalar_min(pinv[:], pinv[:], 1e12)
        phat = singles.tile([K, D], f32)
        nc.vector.tensor_scalar_mul(phat[:], proto_sb[:], pinv[:])
        phatT = singles.tile([128, DT, K], f32)
        for j in range(DT):
            tp = ps1.tile([128, K], f32, tag="ps1")
            nc.tensor.transpose(tp[:], phat[:, j * 128:(j + 1) * 128],
                                ident[0:K, 0:K])
            nc.scalar.activation(phatT[:, j, :], tp[:], F.Copy)

        # persistent state
        gmp = singles.tile([128, CT, BPC], f32)
        attr_all = singles.tile([K, BPC], f32)

        # ---------- Pass A ----------
        for p in range(NPAIR):
            fthi = featp.tile([128, CT, 2, N], bf16, tag="fthi", bufs=2)
            ftlo = featp.tile([128, CT, 2, N], bf16, tag="ftlo", bufs=2)
            for b2 in range(2):
                nc.sync.dma_start(
                    out=fthi[:, :, b2, :],
                    in_=fhi_d[2 * p + b2].rearrange("(ct q) n -> q ct n", q=128))
                nc.sync.dma_start(
                    out=ftlo[:, :, b2, :],
                    in_=flo_d[2 * p + b2].rearrange("(ct q) n -> q ct n", q=128))

            bows = []
            for d in range(DT):
                bps = ps_bow.tile([128, N2], f32, tag="bowps")
                # bias: eb[d-chunk] outer-product ones, then accumulate conv
                nc.tensor.matmul(bps[:], lhsT=ebrow_sb[0:1, d * 128:(d + 1) * 128],
                                 rhs=ones392[:], start=True, stop=False)
                for ci in range(CT):
                    ds_ = slice(d * 128, (d + 1) * 128)
                    nc.tensor.matmul(bps[:], lhsT=whi_sb[:, ci, ds_],
                                     rhs=fthi[:, ci, :, :], start=False,
                                     stop=False)
                    i2 = nc.tensor.matmul(bps[:], lhsT=whi_sb[:, ci, ds_],
                                          rhs=ftlo[:, ci, :, :], start=False,
                                          stop=False)
                    i2.ldweights = False
                    nc.tensor.matmul(bps[:], lhsT=wlo_sb[:, ci, ds_],
                                     rhs=fthi[:, ci, :, :], start=False,
                                     stop=(ci == CT - 1))
                bow_sb = bowp.tile([128, N2], f32, tag="bow")
                nc.scalar.activation(bow_sb[:], bps[:], F.Copy)
                bows.append(bow_sb)

            gps = ps1.tile([K, N2], f32, tag="ps1")
            for d in range(DT):
                nc.tensor.matmul(gps[:], lhsT=phatT[:, d, :], rhs=bows[d][:],
                                 start=(d == 0), stop=(d == DT - 1))
            sps = ps1.tile([1, N2], f32, tag="ps1")
            for d in range(DT):
                sq_sb = scr.tile([128, N2], f32, tag="sq")
                nc.vector.tensor_mul(sq_sb[:], bows[d][:], bows[d][:])
                nc.tensor.matmul(sps[:], lhsT=ones_col[:], rhs=sq_sb[:],
                                 start=(d == 0), stop=(d == DT - 1))
            lssq = small.tile([1, N2], f32, tag="lssq", bufs=2)
            nc.scalar.activation(lssq[:], sps[:], F.Ln)
            rinv = small.tile([1, N2], f32, tag="rinv", bufs=2)
            nc.scalar.activation(rinv[:], lssq[:], F.Exp, scale=-0.5)
            rbc = ps1.tile([K, N2], f32, tag="ps1")
            nc.tensor.matmul(rbc[:], lhsT=ones_row[0:1, 0:K], rhs=rinv[:],
                             start=True, stop=True)
            rbc_sb = scr.tile([K, N2], f32, tag="rbcsb", bufs=1)
            nc.scalar.activation(rbc_sb[:], rbc[:], F.Copy)
            tcos = scr.tile([K, N2], f32, tag="tcos", bufs=1)
            nc.vector.tensor_mul(tcos[:], gps[:], rbc_sb[:])
            kk0 = scr.tile([K, N2], f32, tag="kk0", bufs=1)
            nc.scalar.activation(kk0[:], tcos[:], F.Exp, scale=20.0,
                                 bias=bias_m20[0:K, :])
            KKp = scr.tile([K, N2], f32, tag="KKp", bufs=2)
            nc.vector.tensor_scalar_min(KKp[:], kk0[:], 1.0)
            eps_ = ps1.tile([1, N2], f32, tag="ps1")
            nc.tensor.matmul(eps_[:], lhsT=ones_col[0:K, :], rhs=KKp[:],
                             start=True, stop=True)
            eps_sb = small.tile([1, N2], f32, tag="eps_sb", bufs=2)
            nc.scalar.activation(eps_sb[:], eps_[:], F.Copy)
            Epair = small.tile([2, N], f32, tag="Epair", bufs=2)
            for b2 in range(2):
                nc.gpsimd.dma_start(out=Epair[b2:b2 + 1, :],
                                    in_=eps_sb[0:1, b2 * N:(b2 + 1) * N])

            # --- Sinkhorn: Newton from below, V0 = 0 (per pair, 2 rows) ---
            V = small.tile([2, 1], f32, tag="V")
            nc.vector.memset(V[:], 0.0)
            tE = scr.tile([2, N], f32, tag="tE", bufs=2)
            rec = scr.tile([2, N], f32, tag="rec", bufs=2)
            rscr = scr.tile([2, N], f32, tag="rscr", bufs=2)
            sqd = scr.tile([2, N], f32, tag="sqd", bufs=2)
            for it in range(NEWTON_ITERS):
                nc.vector.tensor_scalar_add(tE[:], Epair[:], V[:])
                nc.vector.reciprocal_approx_accurate(rec[:], tE[:], rscr[:])
                S = small.tile([2, 1], f32, tag="S")
                nc.vector.tensor_reduce(S[:], rec[:], axis=AX.X, op=A.add)
                S2 = small.tile([2, 1], f32, tag="S2")
                nc.vector.affine_mul_reduce(out=sqd[:], accum_out=S2[:], in0=rec[:],
                                            in1=rec[:], scale=1.0, bias=0.0)
                a = small.tile([2, 1], f32, tag="a")
                nc.vector.tensor_mul(a[:], V[:], S2[:])
                den = small.tile([2, 1], f32, tag="den")
                nc.vector.tensor_sub(den[:], S[:], a[:])
                bnum = small.tile([2, 1], f32, tag="bnum")
                nc.vector.tensor_mul(bnum[:], V[:], S[:])
                num = small.tile([2, 1], f32, tag="num")
                nc.vector.tensor_scalar_sub(num[:], bnum[:], CN)
                dinv = small.tile([2, 1], f32, tag="dinv")
                nc.vector.reciprocal(dinv[:], den[:])
                q = small.tile([2, 1], f32, tag="q")
                nc.vector.tensor_mul(q[:], num[:], dinv[:])
                Vn = small.tile([2, 1], f32, tag="V")
                nc.vector.tensor_sub(Vn[:], V[:], q[:])
                V = Vn

            # U = 1/(N*(V+E)), m = 1/MU - (N*V/MU)*U
            NV = small.tile([2, 1], f32, tag="NV")
            nc.vector.tensor_scalar_mul(NV[:], V[:], float(N))
            negNVmu = small.tile([2, 1], f32, tag="negNVmu")
            nc.vector.tensor_scalar_mul(negNVmu[:], NV[:], -1.0 / MU)
            nc.vector.tensor_scalar_add(tE[:], Epair[:], V[:])
            zN = scr.tile([2, N], f32, tag="zN", bufs=2)
            nc.vector.tensor_scalar_mul(zN[:], tE[:], float(N))
            Urow = small.tile([2, N], f32, tag="Urow", bufs=2)
            nc.vector.reciprocal_approx_accurate(Urow[:], zN[:], rscr[:])
            mrow = small.tile([2, N], f32, tag="mrow", bufs=2)
            nc.vector.tensor_scalar(mrow[:], Urow[:], negNVmu[:], 1.0 / MU,
                                    op0=A.mult, op1=A.add)

            # --- Pass C (same pair) ---
            upair = small.tile([1, N2], f32, tag="upair", bufs=2)
            mpair = small.tile([1, N2], f32, tag="mpair", bufs=2)
            for b2 in range(2):
                nc.gpsimd.dma_start(out=upair[0:1, b2 * N:(b2 + 1) * N],
                                    in_=Urow[b2:b2 + 1, :])
                nc.gpsimd.dma_start(out=mpair[0:1, b2 * N:(b2 + 1) * N],
                                    in_=mrow[b2:b2 + 1, :])

            ubc = ps1.tile([K, N2], f32, tag="ps1")
            nc.tensor.matmul(ubc[:], lhsT=ones_row[0:1, 0:K], rhs=upair[:],
                             start=True, stop=True)
            zt = scr.tile([K, N], f32, tag="zt", bufs=1)
            for b2 in range(2):
                nc.vector.affine_mul_reduce(
                    out=zt[:], accum_out=attr_all[:, 2 * p + b2:2 * p + b2 + 1],
                    in0=KKp[:, b2 * N:(b2 + 1) * N],
                    in1=ubc[:, b2 * N:(b2 + 1) * N], scale=1.0, bias=0.0)

            mbc = ps1.tile([128, N2], f32, tag="ps1")
            nc.tensor.matmul(mbc[:], lhsT=ones_row[:], rhs=mpair[:],
                             start=True, stop=True)
            mbc_sb = scr.tile([128, N2], f32, tag="mbcsb")
            nc.scalar.activation(mbc_sb[:], mbc[:], F.Copy)

            yscr = scr.tile([128, N], f32, tag="yscr")
            for b2 in range(2):
                ftc = featp.tile([128, CT, N], f32, tag="featC", bufs=3)
                nc.sync.dma_start(
                    out=ftc[:],
                    in_=feat_d[2 * p + b2].rearrange("(ct q) n -> q ct n", q=128))
                for ci in range(CT):
                    nc.vector._custom_dve(
                        MULMAX, out=yscr[:], in0=ftc[:, ci, :],
                        in1=mbc_sb[:, b2 * N:(b2 + 1) * N],
                        accum_out=gmp[:, ci, 2 * p + b2:2 * p + b2 + 1])

        # ---------- tail ----------
        ghi = singles.tile([128, CT, BPC], bf16)
        nc.scalar.activation(ghi[:], gmp[:], F.Copy)
        gdiff = scr.tile([128, CT, BPC], f32, tag="gdiff", bufs=1)
        nc.vector.tensor_sub(gdiff[:], gmp[:], ghi[:])
        glo = singles.tile([128, CT, BPC], bf16)
        nc.scalar.activation(glo[:], gdiff[:], F.Copy)
        xe_ps = ps1.tile([BPC, D], f32, tag="ps1")
        nc.tensor.matmul(xe_ps[:], lhsT=ones_row[0:1, 0:BPC], rhs=ebrow_sb[:],
                         start=True, stop=False)
        for ci in range(CT):
            nc.tensor.matmul(xe_ps[:], lhsT=ghi[:, ci, :], rhs=whi_sb[:, ci, :],
                             start=False, stop=False)
            j2 = nc.tensor.matmul(xe_ps[:], lhsT=ghi[:, ci, :],
                                  rhs=wlo_sb[:, ci, :], start=False, stop=False)
            j2.ldweights = False
            nc.tensor.matmul(xe_ps[:], lhsT=glo[:, ci, :], rhs=whi_sb[:, ci, :],
                             start=False, stop=(ci == CT - 1))
        xe_sb = singles.tile([BPC, D], f32)
        nc.scalar.activation(xe_sb[:], xe_ps[:], F.Copy)

        # x_emb = l2n(xe)
        sqe = scr.tile([BPC, D], f32, tag="sqe", bufs=1)
        ssqe = small.tile([BPC, 1], f32, tag="ssqe")
        nc.vector.affine_mul_reduce(out=sqe[:], accum_out=ssqe[:], in0=xe_sb[:],
                                    in1=xe_sb[:], scale=1.0, bias=0.0)
        le = small.tile([BPC, 1], f32, tag="le")
        nc.scalar.activation(le[:], ssqe[:], F.Ln)
        einv = small.tile([BPC, 1], f32, tag="einv")
        nc.scalar.activation(einv[:], le[:], F.Exp, scale=-0.5)
        nc.vector.tensor_scalar_min(einv[:], einv[:], 1e12)
        emb_sb = singles.tile([BPC, D], f32)
        nc.vector.tensor_scalar_mul(emb_sb[:], xe_sb[:], einv[:])
        nc.sync.dma_start(out=emb_d, in_=emb_sb[:])

        # x_probs = xe @ cwT + cb
        xeT = singles.tile([128, DT, BPC], f32)
        for j in range(DT):
            tp = ps1.tile([128, BPC], f32, tag="ps1")
            nc.tensor.transpose(tp[:], xe_sb[:, j * 128:(j + 1) * 128],
                                ident[0:BPC, 0:BPC])
            nc.scalar.activation(xeT[:, j, :], tp[:], F.Copy)
        pr_ps = ps1.tile([BPC, NCLS], f32, tag="ps1")
        for j in range(DT):
            nc.tensor.matmul(pr_ps[:], lhsT=xeT[:, j, :], rhs=cwT_sb[:, j, :],
                             start=(j == 0), stop=False)
        nc.tensor.matmul(pr_ps[:], lhsT=ones_row[0:1, 0:BPC], rhs=cbrow_sb[:],
                         start=False, stop=True)
        pr_sb = singles.tile([BPC, NCLS], f32)
        nc.scalar.activation(pr_sb[:], pr_ps[:], F.Copy)
        nc.sync.dma_start(out=probs_d, in_=pr_sb[:])

        # x_attr: scale by 1/MU, transpose [K, BPC] -> [BPC, K]
        attr_s = scr.tile([K, BPC], f32, tag="attr_s", bufs=1)
        nc.vector.tensor_scalar_mul(attr_s[:], attr_all[:], 1.0 / MU)
        at_ps = ps1.tile([BPC, K], f32, tag="ps1")
        nc.tensor.transpose(at_ps[:], attr_s[:], ident[0:K, 0:K])
        at_sb = singles.tile([BPC, K], f32)
        nc.scalar.activation(at_sb[:], at_ps[:], F.Copy)
        nc.sync.dma_start(out=attr_d, in_=at_sb[:])

    nc.compile()
    return nc


def kernel(features, embed_w, embed_b, cls_w, cls_b, prototypes):
    from concourse.bass_utils import run_bass_kernel_spmd

    feats = np.ascontiguousarray(np.asarray(features, np.float32).reshape(B, C, N))
    ew = np.asarray(embed_w, np.float32)
    eb = np.asarray(embed_b, np.float32)
    cw = np.asarray(cls_w, np.float32)
    cb = np.asarray(cls_b, np.float32)
    pt = np.ascontiguousarray(np.asarray(prototypes, np.float32))

    import ml_dtypes
    bf = ml_dtypes.bfloat16
    ewT = np.ascontiguousarray(ew.T)                      # (C, D)
    ewThi = ewT.astype(bf)
    ewTlo = (ewT - ewThi.astype(np.float32)).astype(bf)
    fhi = feats.astype(bf)
    flo = (feats - fhi.astype(np.float32)).astype(bf)
    cwT = np.ascontiguousarray(cw.T)                      # (D, NCLS)
    ebrow = np.ascontiguousarray(eb.reshape(1, D))
    cbrow = np.ascontiguousarray(cb.reshape(1, NCLS))

    if "nc" not in _cache:
        _cache["nc"] = _build_nc()
    nc = _cache["nc"]

    in_maps = []
    for i in range(NCORES):
        in_maps.append({
            "features": np.ascontiguousarray(feats[i * BPC:(i + 1) * BPC]),
            "fhi": np.ascontiguousarray(fhi[i * BPC:(i + 1) * BPC]),
            "flo": np.ascontiguousarray(flo[i * BPC:(i + 1) * BPC]),
            "ewThi": ewThi, "ewTlo": ewTlo, "ebrow": ebrow,
            "cwT": cwT, "cbrow": cbrow, "prototypes": pt,
        })

    trace = bool(os.environ.get("KERNEL_TRACE"))
    res = run_bass_kernel_spmd(nc, in_maps, core_ids=list(range(NCORES)),
                               trace=trace)
    if trace and res.exec_time_ns is not None:
        print(f"HW exec time: {res.exec_time_ns} ns")
        _cache["exec_time_ns"] = res.exec_time_ns
        _cache["results_obj"] = res

    probs = np.concatenate([res.results[i]["x_probs"] for i in range(NCORES)], 0)
    emb = np.concatenate([res.results[i]["x_emb"] for i in range(NCORES)], 0)
    attr = np.concatenate([res.results[i]["x_attr"] for i in range(NCORES)], 0)
    return probs, emb, attr
